# revision 33
# baseline (speedup 1.0000x reference)
"""Multi-head attention Bass kernel for Trainium2, sharded over 8 NeuronCores.

Problem: B=2, S=2048, D=768, H=12 heads (d_k=64). Returns (output, attention_weights).

Sharding (data + head parallel): core c handles batch b = c//4 and heads
h0 = (c%4)*3 .. h0+3 (3 heads). W_q/W_k/W_v are split column-wise, W_o row-wise
over heads. Each core computes its 3 heads' [S,S] attention weights and a partial
output projection; the host sums the 4 partial outputs per batch and re-transposes
the attention weights.

Device-side layout: everything is computed in transposed orientation
(scores^T = [k, q]) so the second attention matmul (P @ V) needs no on-chip
transposes: lhsT = V_aug [k, d+1] (ones column appended -> sum(exp) for free),
rhs = E^T [k, q]. Attention weights are written to DRAM as P^T [h, k, q] and
un-transposed on the host during gather.

The scores matmuls contract over d_k=64 (half the PE array), so two heads'
scores run CONCURRENTLY via row-group packing (lhsT base partitions 0 / 64).
Heads are processed as unit0 = head0 (solo, minimizes time to first output)
then unit1 = head1+head2 packed.

Precision: matmul inputs are bf16 (PE full rate); accumulation is fp32 in PSUM;
softmax (exp, reciprocal, normalize) is fp32; outputs are fp32.
"""
import numpy as np
import ml_dtypes

import concourse.bass as bass
import concourse.tile as tile
from concourse import bacc, mybir, bass_utils

F32 = mybir.dt.float32
BF16 = mybir.dt.bfloat16
AF = mybir.ActivationFunctionType

B = 2
S = 2048
D = 768
H_TOT = 12
DK = 64
H = 3            # heads per core
N_CORES = 8
ST = S // 128    # 16 seq tiles
DT = D // 128    # 6 d-model tiles
QH = 1024        # q-half size
NQH = S // QH    # 2
SCALE = 1.0 / np.sqrt(DK)

_CACHED_NC = None


def build_nc():
    nc = bacc.Bacc("TRN2", target_bir_lowering=False, debug=False, num_devices=N_CORES)

    xq = nc.dram_tensor("xq", [D, S], BF16, kind="ExternalInput").ap()  # x^T, host-prep
    xk = nc.dram_tensor("xk", [D, S], BF16, kind="ExternalInput").ap()
    xv = nc.dram_tensor("xv", [D, S], BF16, kind="ExternalInput").ap()
    wq = nc.dram_tensor("wq", [D, H * DK], BF16, kind="ExternalInput").ap()
    wk = nc.dram_tensor("wk", [D, H * DK], BF16, kind="ExternalInput").ap()
    wv = nc.dram_tensor("wv", [D, H * DK], BF16, kind="ExternalInput").ap()
    wo = nc.dram_tensor("wo", [H * DK, D], BF16, kind="ExternalInput").ap()
    bo = nc.dram_tensor("bo", [D], F32, kind="ExternalInput").ap()

    pt = nc.dram_tensor("pt", [H, NQH, S, QH], F32, kind="ExternalOutput").ap()
    yt = nc.dram_tensor("yt", [D, S], F32, kind="ExternalOutput").ap()

    with tile.TileContext(nc) as tc:
        _emit(nc, tc, xq, xk, xv, wq, wk, wv, wo, bo, pt, yt)
    nc.compile()
    return nc


def _emit(nc, tc, xq, xk, xv, wq, wk, wv, wo, bo, pt, yt):
    from contextlib import ExitStack

    ctx = ExitStack()
    singles = ctx.enter_context(tc.tile_pool(name="singles", bufs=1))
    # Q^T / K^T head-packed tiles: tile 0 = heads 0,1 (partitions 0-63 / 64-127),
    # tile 1 = head 2 (partitions 0-63).
    qkt_pool = ctx.enter_context(tc.tile_pool(name="qkt", bufs=1))
    vaug_pool = ctx.enter_context(tc.tile_pool(name="vaug", bufs=1))
    an_pool = ctx.enter_context(tc.tile_pool(name="an", bufs=1))
    et_pool = ctx.enter_context(tc.tile_pool(name="et", bufs=1))
    small_pool = ctx.enter_context(tc.tile_pool(name="small", bufs=2))
    rbc_pool = ctx.enter_context(tc.tile_pool(name="rbc", bufs=3))
    rdram_pool = ctx.enter_context(tc.tile_pool(name="rdram", bufs=3, space="DRAM"))
    ptn_pool = ctx.enter_context(tc.tile_pool(name="ptn", bufs=1))
    eta_pool = ctx.enter_context(tc.tile_pool(name="eta", bufs=1))
    y_pool = ctx.enter_context(tc.tile_pool(name="ysb", bufs=2))
    ps_s = ctx.enter_context(tc.tile_pool(name="ps_s", bufs=2, space="PSUM"))
    ps_o = ctx.enter_context(tc.tile_pool(name="ps_o", bufs=2, space="PSUM"))

    ones_bf = singles.tile([128, 1], BF16)
    nc.vector.memset(ones_bf[:], 1.0)
    wo12_t = singles.tile([128, D], BF16)      # W_o rows 64..191 (heads 1,2)
    nc.sync.dma_start(wo12_t[:], wo[64 : H * DK, :])
    wo0_t = singles.tile([64, D], BF16)        # W_o rows 0..63 (head 0)
    nc.sync.dma_start(wo0_t[:], wo[0:64, :])
    bo_t = singles.tile([128, DT], F32)
    nc.sync.dma_start(bo_t[:], bo.rearrange("(t p) -> p t", p=128))

    qt_tiles = [qkt_pool.tile([128, S], BF16, tag=f"qt{m}", name=f"qt{m}") for m in range(2)]
    kt_tiles = [qkt_pool.tile([128, S], BF16, tag=f"kt{m}", name=f"kt{m}") for m in range(2)]
    vaug = [vaug_pool.tile([128, H, DK + 1], BF16, tag=f"va{m}", name=f"va{m}") for m in range(ST)]
    # an12: heads 1 (parts 0-63) and 2 (parts 64-127) packed; an0: head 0.
    an12 = [an_pool.tile([128, QH], BF16, tag=f"an12_{q}", name=f"an12_{q}") for q in range(NQH)]
    an0 = [an_pool.tile([DK, QH], BF16, tag=f"an0_{q}", name=f"an0_{q}") for q in range(NQH)]

    # head -> (tile, base partition) in the packed Q^T/K^T tiles
    HSLOT = {0: (0, 0), 1: (0, 64), 2: (1, 0)}

    # ---------------- Phase A: loads + projections + first unit ----------------
    def make_attn_unit(et_lookup):
        def attn_unit(qh, heads, lanes):
            o_ps = {}
            for h in heads:
                o_ps[h] = ps_o.tile([DK + 1, QH], F32, tag="ops", name=f"o_ps{h}")
            ets = {h: [] for h in heads}
            for kt in range(ST):
                s_tiles = {}
                for h in heads:
                    s_tiles[h] = ps_s.tile([128, QH], F32, tag="sps", name=f"s_ps{h}")
                for j in range(QH // 512):
                    for h in heads:
                        ti, p0 = HSLOT[h]
                        nc.tensor.matmul(
                            s_tiles[h][:, j * 512 : (j + 1) * 512],
                            kt_tiles[ti][p0 : p0 + DK, kt * 128 : (kt + 1) * 128],
                            qt_tiles[ti][p0 : p0 + DK,
                                         qh * QH + j * 512 : qh * QH + (j + 1) * 512],
                            start=True,
                            stop=True,
                            tile_position=(p0, 0),
                        )
                for i, h in enumerate(heads):
                    pool, lane = et_lookup(lanes[i])
                    et = pool.tile([128, QH], BF16, tag=f"et{lane}_{kt}",
                                   name=f"et{lane}_{kt}")
                    ets[h].append(et)
                    nc.scalar.activation(et[:], s_tiles[h][:], AF.Exp, scale=float(SCALE))
                for h in heads:
                    et = ets[h][kt]
                    for j in range(QH // 512):
                        nc.tensor.matmul(
                            o_ps[h][:, j * 512 : (j + 1) * 512],
                            vaug[kt][:, h, :],
                            et[:, j * 512 : (j + 1) * 512],
                            start=(kt == 0),
                            stop=(kt == ST - 1),
                        )

            rbcs = {}
            for h in heads:
                # free o_ps fast via ACT copy; recip spread over 128 partitions;
                # all small DMAs on gpsimd SWDGE (off the P^T sync FIFO)
                o_sb = small_pool.tile([DK + 1, QH], F32, tag="osb", name="o_sb")
                nc.scalar.copy(o_sb[:], o_ps[h][:])
                r_d = rdram_pool.tile([1, QH], F32, tag="rd", name="r_d")
                nc.gpsimd.dma_start(r_d[:], o_sb[DK : DK + 1, :])
                rs8 = small_pool.tile([128, QH // 128], F32, tag="rs8", name="rs8")
                nc.gpsimd.dma_start(rs8[:], r_d[:].rearrange("a (p e) -> (a p) e", p=128))
                nc.vector.reciprocal(rs8[:], rs8[:])
                r2_d = rdram_pool.tile([1, QH], F32, tag="rd2", name="r2_d")
                nc.gpsimd.dma_start(r2_d[:].rearrange("a (p e) -> (a p) e", p=128), rs8[:])
                rbc = rbc_pool.tile([128, QH], F32, tag="rbc", name="rbc")
                nc.gpsimd.dma_start(rbc[:], r2_d[:].to_broadcast((128, QH)))
                rbcs[h] = rbc

                if h == 0:
                    an_dst = an0[qh][:]
                else:
                    an_dst = an12[qh][(h - 1) * DK : h * DK, :]
                nc.vector.tensor_mul(an_dst, o_sb[0:DK, :], rbc[0:DK, :])

            for h in heads:
                rbc = rbcs[h]
                for kt in range(ST):
                    ptt = ptn_pool.tile([128, QH], F32, tag=f"ptn{kt % 3}",
                                        name=f"ptn_{kt % 3}")
                    if kt % 2 == 1:
                        nc.gpsimd.tensor_mul(ptt[:], ets[h][kt][:], rbc[:])
                    else:
                        nc.vector.tensor_mul(ptt[:], ets[h][kt][:], rbc[:])
                    nc.sync.dma_start(pt[h, qh, kt * 128 : (kt + 1) * 128, :], ptt[:])
        return attn_unit

    with tc.tile_pool(name="xt", bufs=2) as xt_pool, \
         tc.tile_pool(name="wqkv", bufs=1) as w_pool:
        wq_t = w_pool.tile([128, DT, H * DK], BF16)
        wk_t = w_pool.tile([128, DT, H * DK], BF16)
        wv_t = w_pool.tile([128, DT, H * DK], BF16)
        nc.sync.dma_start(wq_t[:], wq.rearrange("(t p) n -> p t n", p=128))
        nc.sync.dma_start(wk_t[:], wk.rearrange("(t p) n -> p t n", p=128))
        nc.sync.dma_start(wv_t[:], wv.rearrange("(t p) n -> p t n", p=128))

        xts = {}
        for which, xin in ((2, xv), (0, xq), (1, xk)):
            xin_r = xin.rearrange("(t p) s -> p t s", p=128)
            xt = xt_pool.tile([128, DT, S], BF16, tag="xt", name=f"xt{which}")
            xts[which] = xt
            for dt in range(DT):
                nc.sync.dma_start(xt[:, dt, :], xin_r[:, dt, :])

        # ---- V projection -> V_aug [k, (h, d+1)] with ones column ----
        for m in range(ST):
            pv = ps_s.tile([128, QH], F32, tag="sps", name="pv")
            for dt in range(DT):
                nc.tensor.matmul(
                    pv[:, 0 : H * DK],
                    xts[2][:, dt, m * 128 : (m + 1) * 128],
                    wv_t[:, dt, :],
                    start=(dt == 0),
                    stop=(dt == DT - 1),
                )
            va = vaug[m]
            nc.vector.tensor_copy(
                va[:, :, 0:DK],
                pv[:, 0 : H * DK].rearrange("p (h d) -> p h d", h=H),
            )
            nc.vector.tensor_copy(va[:, :, DK : DK + 1], ones_bf[:].to_broadcast((128, H, 1)))

        def project(which, m):
            xt = xts[which]
            w_t = wq_t if which == 0 else wk_t
            dst = (qt_tiles if which == 0 else kt_tiles)[m]
            cols = slice(m * 128, m * 128 + (128 if m == 0 else 64))
            npart = 128 if m == 0 else 64
            for jh in range(NQH):
                pp = ps_s.tile([128, QH], F32, tag="sps", name="pp")
                for dt in range(DT):
                    for j in range(QH // 512):
                        nc.tensor.matmul(
                            pp[:npart, j * 512 : (j + 1) * 512],
                            w_t[:, dt, cols],
                            xt[:, dt, jh * QH + j * 512 : jh * QH + (j + 1) * 512],
                            start=(dt == 0),
                            stop=(dt == DT - 1),
                        )
                nc.vector.tensor_copy(dst[:npart, jh * QH : (jh + 1) * QH], pp[:npart, :])

        project(0, 0)
        project(1, 0)

        attn_unit_a = make_attn_unit(lambda lane: (eta_pool, lane))
        attn_unit_a(0, [0], ["a"])

        project(0, 1)
        project(1, 1)

    # ---------------- Phase B: remaining units + output projection -------------
    with tc.tile_pool(name="et", bufs=1) as et_pool:
        attn_unit = make_attn_unit(
            lambda lane: (eta_pool, lane) if lane == "a" else (et_pool, lane)
        )

        def emit_y(qh):
            for dt in range(DT):
                y_ps = ps_o.tile([128, QH], F32, tag="ops", name="y_ps")
                for j in range(QH // 512):
                    js = slice(j * 512, (j + 1) * 512)
                    nc.tensor.matmul(
                        y_ps[:, js],
                        wo12_t[:, dt * 128 : (dt + 1) * 128],
                        an12[qh][:, js],
                        start=True,
                        stop=False,
                    )
                    nc.tensor.matmul(
                        y_ps[:, js],
                        wo0_t[:, dt * 128 : (dt + 1) * 128],
                        an0[qh][:, js],
                        start=False,
                        stop=True,
                    )
                y_sb = y_pool.tile([128, QH], F32, tag="y", name="y_sb")
                nc.scalar.activation(
                    y_sb[:], y_ps[:], AF.Identity, bias=bo_t[:, dt : dt + 1], scale=1.0
                )
                nc.scalar.dma_start(
                    yt[dt * 128 : (dt + 1) * 128, qh * QH : (qh + 1) * QH], y_sb[:]
                )

        attn_unit(0, [1, 2], [0, 1])
        attn_unit(1, [1, 2], ["a", 2])
        emit_y(0)
        attn_unit(1, [0], [1])
        emit_y(1)

    ctx.close()


def _make_in_maps(query, key, value, W_q, W_k, W_v, W_o, b_o):
    bf = ml_dtypes.bfloat16
    in_maps = []
    for c in range(N_CORES):
        b = c // 4
        h0 = (c % 4) * H
        cols = slice(h0 * DK, (h0 + H) * DK)
        in_maps.append(
            {
                "xq": np.ascontiguousarray(query[b].T).astype(bf),
                "xk": np.ascontiguousarray(key[b].T).astype(bf),
                "xv": np.ascontiguousarray(value[b].T).astype(bf),
                "wq": np.ascontiguousarray(W_q[:, cols]).astype(bf),
                "wk": np.ascontiguousarray(W_k[:, cols]).astype(bf),
                "wv": np.ascontiguousarray(W_v[:, cols]).astype(bf),
                "wo": np.ascontiguousarray(W_o[cols, :]).astype(bf),
                "bo": np.ascontiguousarray(b_o) / 4.0,
            }
        )
    return in_maps


def run_traced(inputs):
    """Run with NTFF tracing to get HW exec time (test-only helper)."""
    nc = _get_nc()
    in_maps = _make_in_maps(
        inputs["query"], inputs["key"], inputs["value"],
        inputs["W_q"], inputs["W_k"], inputs["W_v"], inputs["W_o"], inputs["b_o"],
    )
    return bass_utils.run_bass_kernel_spmd(
        nc, in_maps, core_ids=list(range(N_CORES)), trace=True
    )


def _get_nc():
    global _CACHED_NC
    if _CACHED_NC is None:
        _CACHED_NC = build_nc()
    return _CACHED_NC


def kernel(query, key, value, W_q, W_k, W_v, W_o, b_o):
    query = np.asarray(query, dtype=np.float32)
    key = np.asarray(key, dtype=np.float32)
    value = np.asarray(value, dtype=np.float32)
    W_q = np.asarray(W_q, dtype=np.float32)
    W_k = np.asarray(W_k, dtype=np.float32)
    W_v = np.asarray(W_v, dtype=np.float32)
    W_o = np.asarray(W_o, dtype=np.float32)
    b_o = np.asarray(b_o, dtype=np.float32)

    nc = _get_nc()
    in_maps = _make_in_maps(query, key, value, W_q, W_k, W_v, W_o, b_o)
    res = bass_utils.run_bass_kernel_spmd(nc, in_maps, core_ids=list(range(N_CORES)))

    attn = np.empty((B, H_TOT, S, S), dtype=np.float32)
    out = np.zeros((B, S, D), dtype=np.float32)
    for c in range(N_CORES):
        b = c // 4
        h0 = (c % 4) * H
        r = res.results[c]
        ptc = r["pt"]  # [H, NQH, S(k), QH(q)]
        for j in range(H):
            attn[b, h0 + j] = np.concatenate([ptc[j, q] for q in range(NQH)], axis=1).T
        out[b] += r["yt"].T  # [D, S] -> [S, D]
    return out, attn


# revision 34
# speedup vs baseline: 1.0691x; 1.0691x over previous
"""Multi-head attention Bass kernel for Trainium2, sharded over 8 NeuronCores.

Problem: B=2, S=2048, D=768, H=12 heads (d_k=64). Returns (output, attention_weights).

Sharding (data + head parallel): core c handles batch b = c//4 and heads
h0 = (c%4)*3 .. h0+3 (3 heads). W_q/W_k/W_v are split column-wise, W_o row-wise
over heads. Each core computes its 3 heads' [S,S] attention weights and a partial
output projection; the host sums the 4 partial outputs per batch and re-transposes
the attention weights.

Device-side layout: everything is computed in transposed orientation
(scores^T = [k, q]) so the second attention matmul (P @ V) needs no on-chip
transposes: lhsT = V_aug [k, d+1] (ones column appended -> sum(exp) for free),
rhs = E^T [k, q]. Attention weights are written to DRAM as P^T [h, k, q] and
un-transposed on the host during gather.

The scores matmuls contract over d_k=64 (half the PE array), so two heads'
scores run CONCURRENTLY via row-group packing (lhsT base partitions 0 / 64).
Heads are processed as unit0 = head0 (solo, minimizes time to first output)
then unit1 = head1+head2 packed.

Precision: matmul inputs are bf16 (PE full rate); accumulation is fp32 in PSUM;
softmax (exp, reciprocal, normalize) is fp32; outputs are fp32.
"""
import numpy as np
import ml_dtypes

import concourse.bass as bass
import concourse.tile as tile
from concourse import bacc, mybir, bass_utils

F32 = mybir.dt.float32
BF16 = mybir.dt.bfloat16
AF = mybir.ActivationFunctionType

B = 2
S = 2048
D = 768
H_TOT = 12
DK = 64
H = 3            # heads per core
N_CORES = 8
ST = S // 128    # 16 seq tiles
DT = D // 128    # 6 d-model tiles
QH = 1024        # q-half size
NQH = S // QH    # 2
SCALE = 1.0 / np.sqrt(DK)

_CACHED_NC = None


def build_nc():
    nc = bacc.Bacc("TRN2", target_bir_lowering=False, debug=False, num_devices=N_CORES)

    xq = nc.dram_tensor("xq", [D, S], BF16, kind="ExternalInput").ap()  # x^T, host-prep
    xk = nc.dram_tensor("xk", [D, S], BF16, kind="ExternalInput").ap()
    xv = nc.dram_tensor("xv", [D, S], BF16, kind="ExternalInput").ap()
    wq = nc.dram_tensor("wq", [D, H * DK], BF16, kind="ExternalInput").ap()
    wk = nc.dram_tensor("wk", [D, H * DK], BF16, kind="ExternalInput").ap()
    wv = nc.dram_tensor("wv", [D, H * DK], BF16, kind="ExternalInput").ap()
    wo = nc.dram_tensor("wo", [H * DK, D], BF16, kind="ExternalInput").ap()
    bo = nc.dram_tensor("bo", [D], F32, kind="ExternalInput").ap()

    pt = nc.dram_tensor("pt", [H, NQH, S, QH], F32, kind="ExternalOutput").ap()
    yt = nc.dram_tensor("yt", [D, S], F32, kind="ExternalOutput").ap()

    with tile.TileContext(nc) as tc:
        _emit(nc, tc, xq, xk, xv, wq, wk, wv, wo, bo, pt, yt)
    nc.compile()
    return nc


def _emit(nc, tc, xq, xk, xv, wq, wk, wv, wo, bo, pt, yt):
    from contextlib import ExitStack

    ctx = ExitStack()
    singles = ctx.enter_context(tc.tile_pool(name="singles", bufs=1))
    # Q^T / K^T head-packed tiles: tile 0 = heads 0,1 (partitions 0-63 / 64-127),
    # tile 1 = head 2 (partitions 0-63).
    qkt_pool = ctx.enter_context(tc.tile_pool(name="qkt", bufs=1))
    vaug_pool = ctx.enter_context(tc.tile_pool(name="vaug", bufs=1))
    an_pool = ctx.enter_context(tc.tile_pool(name="an", bufs=1))
    et_pool = ctx.enter_context(tc.tile_pool(name="et", bufs=1))
    small_pool = ctx.enter_context(tc.tile_pool(name="small", bufs=2))
    rbc_pool = ctx.enter_context(tc.tile_pool(name="rbc", bufs=3))
    rdram_pool = ctx.enter_context(tc.tile_pool(name="rdram", bufs=3, space="DRAM"))
    ptn_pool = ctx.enter_context(tc.tile_pool(name="ptn", bufs=1))
    eta_pool = ctx.enter_context(tc.tile_pool(name="eta", bufs=1))
    y_pool = ctx.enter_context(tc.tile_pool(name="ysb", bufs=2))
    ps_s = ctx.enter_context(tc.tile_pool(name="ps_s", bufs=2, space="PSUM"))
    ps_o = ctx.enter_context(tc.tile_pool(name="ps_o", bufs=2, space="PSUM"))

    ones_bf = singles.tile([128, 1], BF16)
    nc.vector.memset(ones_bf[:], 1.0)
    wo12_t = singles.tile([128, D], BF16)      # W_o rows 64..191 (heads 1,2)
    nc.sync.dma_start(wo12_t[:], wo[64 : H * DK, :])
    wo0_t = singles.tile([64, D], BF16)        # W_o rows 0..63 (head 0)
    nc.sync.dma_start(wo0_t[:], wo[0:64, :])
    bo_t = singles.tile([128, DT], F32)
    nc.sync.dma_start(bo_t[:], bo.rearrange("(t p) -> p t", p=128))

    qt_tiles = [qkt_pool.tile([128, S], BF16, tag=f"qt{m}", name=f"qt{m}") for m in range(2)]
    kt_tiles = [qkt_pool.tile([128, S], BF16, tag=f"kt{m}", name=f"kt{m}") for m in range(2)]
    vaug = [vaug_pool.tile([128, H, DK + 1], BF16, tag=f"va{m}", name=f"va{m}") for m in range(ST)]
    # an12: heads 1 (parts 0-63) and 2 (parts 64-127) packed; an0: head 0.
    an12 = [an_pool.tile([128, QH], BF16, tag=f"an12_{q}", name=f"an12_{q}") for q in range(NQH)]
    an0 = [an_pool.tile([DK, QH], BF16, tag=f"an0_{q}", name=f"an0_{q}") for q in range(NQH)]

    # head -> (tile, base partition) in the packed Q^T/K^T tiles
    HSLOT = {0: (0, 0), 1: (0, 64), 2: (1, 0)}

    # ---------------- Phase A: loads + projections + first unit ----------------
    def make_attn_unit(et_lookup):
        def attn_unit(qh, heads, lanes):
            o_ps = {}
            for h in heads:
                o_ps[h] = ps_o.tile([DK + 1, QH], F32, tag="ops", name=f"o_ps{h}")
            ets = {h: [] for h in heads}
            for kt in range(ST):
                s_tiles = {}
                for h in heads:
                    s_tiles[h] = ps_s.tile([128, QH], F32, tag="sps", name=f"s_ps{h}")
                for j in range(QH // 512):
                    for h in heads:
                        ti, p0 = HSLOT[h]
                        nc.tensor.matmul(
                            s_tiles[h][:, j * 512 : (j + 1) * 512],
                            kt_tiles[ti][p0 : p0 + DK, kt * 128 : (kt + 1) * 128],
                            qt_tiles[ti][p0 : p0 + DK,
                                         qh * QH + j * 512 : qh * QH + (j + 1) * 512],
                            start=True,
                            stop=True,
                            tile_position=(p0, 0),
                        )
                for i, h in enumerate(heads):
                    pool, lane = et_lookup(lanes[i])
                    et = pool.tile([128, QH], BF16, tag=f"et{lane}_{kt}",
                                   name=f"et{lane}_{kt}")
                    ets[h].append(et)
                    nc.scalar.activation(et[:], s_tiles[h][:], AF.Exp, scale=float(SCALE))
                for h in heads:
                    et = ets[h][kt]
                    for j in range(QH // 512):
                        nc.tensor.matmul(
                            o_ps[h][:, j * 512 : (j + 1) * 512],
                            vaug[kt][:, h, :],
                            et[:, j * 512 : (j + 1) * 512],
                            start=(kt == 0),
                            stop=(kt == ST - 1),
                        )

            rbcs = {}
            for h in heads:
                # free o_ps fast via ACT copy; recip spread over 128 partitions;
                # all small DMAs on gpsimd SWDGE (off the P^T sync FIFO)
                o_sb = small_pool.tile([DK + 1, QH], F32, tag="osb", name="o_sb")
                nc.scalar.copy(o_sb[:], o_ps[h][:])
                r_d = rdram_pool.tile([1, QH], F32, tag="rd", name="r_d")
                nc.sync.dma_start(r_d[:], o_sb[DK : DK + 1, :])
                rs8 = small_pool.tile([128, QH // 128], F32, tag="rs8", name="rs8")
                nc.sync.dma_start(rs8[:], r_d[:].rearrange("a (p e) -> (a p) e", p=128))
                nc.vector.reciprocal(rs8[:], rs8[:])
                r2_d = rdram_pool.tile([1, QH], F32, tag="rd2", name="r2_d")
                nc.sync.dma_start(r2_d[:].rearrange("a (p e) -> (a p) e", p=128), rs8[:])
                rbc = rbc_pool.tile([128, QH], F32, tag="rbc", name="rbc")
                nc.sync.dma_start(rbc[:], r2_d[:].to_broadcast((128, QH)))
                rbcs[h] = rbc

                if h == 0:
                    an_dst = an0[qh][:]
                else:
                    an_dst = an12[qh][(h - 1) * DK : h * DK, :]
                nc.vector.tensor_mul(an_dst, o_sb[0:DK, :], rbc[0:DK, :])

            for h in heads:
                rbc = rbcs[h]
                for kt in range(ST):
                    ptt = ptn_pool.tile([128, QH], F32, tag=f"ptn{kt % 3}",
                                        name=f"ptn_{kt % 3}")
                    if kt % 3 == 2:
                        nc.gpsimd.tensor_mul(ptt[:], ets[h][kt][:], rbc[:])
                    else:
                        nc.vector.tensor_mul(ptt[:], ets[h][kt][:], rbc[:])
                    nc.sync.dma_start(pt[h, qh, kt * 128 : (kt + 1) * 128, :], ptt[:])
        return attn_unit

    with tc.tile_pool(name="xt", bufs=2) as xt_pool, \
         tc.tile_pool(name="wqkv", bufs=1) as w_pool:
        wq_t = w_pool.tile([128, DT, H * DK], BF16)
        wk_t = w_pool.tile([128, DT, H * DK], BF16)
        wv_t = w_pool.tile([128, DT, H * DK], BF16)
        nc.sync.dma_start(wq_t[:], wq.rearrange("(t p) n -> p t n", p=128))
        nc.sync.dma_start(wk_t[:], wk.rearrange("(t p) n -> p t n", p=128))
        nc.sync.dma_start(wv_t[:], wv.rearrange("(t p) n -> p t n", p=128))

        xts = {}
        for which, xin in ((2, xv), (0, xq), (1, xk)):
            xin_r = xin.rearrange("(t p) s -> p t s", p=128)
            xt = xt_pool.tile([128, DT, S], BF16, tag="xt", name=f"xt{which}")
            xts[which] = xt
            for dt in range(DT):
                nc.sync.dma_start(xt[:, dt, :], xin_r[:, dt, :])

        # ---- V projection -> V_aug [k, (h, d+1)] with ones column ----
        for m in range(ST):
            pv = ps_s.tile([128, QH], F32, tag="sps", name="pv")
            for dt in range(DT):
                nc.tensor.matmul(
                    pv[:, 0 : H * DK],
                    xts[2][:, dt, m * 128 : (m + 1) * 128],
                    wv_t[:, dt, :],
                    start=(dt == 0),
                    stop=(dt == DT - 1),
                )
            va = vaug[m]
            nc.vector.tensor_copy(
                va[:, :, 0:DK],
                pv[:, 0 : H * DK].rearrange("p (h d) -> p h d", h=H),
            )
            nc.vector.tensor_copy(va[:, :, DK : DK + 1], ones_bf[:].to_broadcast((128, H, 1)))

        def project(which, m):
            xt = xts[which]
            w_t = wq_t if which == 0 else wk_t
            dst = (qt_tiles if which == 0 else kt_tiles)[m]
            cols = slice(m * 128, m * 128 + (128 if m == 0 else 64))
            npart = 128 if m == 0 else 64
            for jh in range(NQH):
                pp = ps_s.tile([128, QH], F32, tag="sps", name="pp")
                for dt in range(DT):
                    for j in range(QH // 512):
                        nc.tensor.matmul(
                            pp[:npart, j * 512 : (j + 1) * 512],
                            w_t[:, dt, cols],
                            xt[:, dt, jh * QH + j * 512 : jh * QH + (j + 1) * 512],
                            start=(dt == 0),
                            stop=(dt == DT - 1),
                        )
                nc.vector.tensor_copy(dst[:npart, jh * QH : (jh + 1) * QH], pp[:npart, :])

        project(0, 0)
        project(1, 0)

        attn_unit_a = make_attn_unit(lambda lane: (eta_pool, lane))
        attn_unit_a(0, [0], ["a"])

        project(0, 1)
        project(1, 1)

    # ---------------- Phase B: remaining units + output projection -------------
    with tc.tile_pool(name="et", bufs=1) as et_pool:
        attn_unit = make_attn_unit(
            lambda lane: (eta_pool, lane) if lane == "a" else (et_pool, lane)
        )

        def emit_y(qh):
            for dt in range(DT):
                y_ps = ps_o.tile([128, QH], F32, tag="ops", name="y_ps")
                for j in range(QH // 512):
                    js = slice(j * 512, (j + 1) * 512)
                    nc.tensor.matmul(
                        y_ps[:, js],
                        wo12_t[:, dt * 128 : (dt + 1) * 128],
                        an12[qh][:, js],
                        start=True,
                        stop=False,
                    )
                    nc.tensor.matmul(
                        y_ps[:, js],
                        wo0_t[:, dt * 128 : (dt + 1) * 128],
                        an0[qh][:, js],
                        start=False,
                        stop=True,
                    )
                y_sb = y_pool.tile([128, QH], F32, tag="y", name="y_sb")
                nc.scalar.activation(
                    y_sb[:], y_ps[:], AF.Identity, bias=bo_t[:, dt : dt + 1], scale=1.0
                )
                nc.scalar.dma_start(
                    yt[dt * 128 : (dt + 1) * 128, qh * QH : (qh + 1) * QH], y_sb[:]
                )

        attn_unit(0, [1, 2], [0, 1])
        attn_unit(1, [1, 2], ["a", 2])
        emit_y(0)
        attn_unit(1, [0], [1])
        emit_y(1)

    ctx.close()


def _make_in_maps(query, key, value, W_q, W_k, W_v, W_o, b_o):
    bf = ml_dtypes.bfloat16
    in_maps = []
    for c in range(N_CORES):
        b = c // 4
        h0 = (c % 4) * H
        cols = slice(h0 * DK, (h0 + H) * DK)
        in_maps.append(
            {
                "xq": np.ascontiguousarray(query[b].T).astype(bf),
                "xk": np.ascontiguousarray(key[b].T).astype(bf),
                "xv": np.ascontiguousarray(value[b].T).astype(bf),
                "wq": np.ascontiguousarray(W_q[:, cols]).astype(bf),
                "wk": np.ascontiguousarray(W_k[:, cols]).astype(bf),
                "wv": np.ascontiguousarray(W_v[:, cols]).astype(bf),
                "wo": np.ascontiguousarray(W_o[cols, :]).astype(bf),
                "bo": np.ascontiguousarray(b_o) / 4.0,
            }
        )
    return in_maps


def run_traced(inputs):
    """Run with NTFF tracing to get HW exec time (test-only helper)."""
    nc = _get_nc()
    in_maps = _make_in_maps(
        inputs["query"], inputs["key"], inputs["value"],
        inputs["W_q"], inputs["W_k"], inputs["W_v"], inputs["W_o"], inputs["b_o"],
    )
    return bass_utils.run_bass_kernel_spmd(
        nc, in_maps, core_ids=list(range(N_CORES)), trace=True
    )


def _get_nc():
    global _CACHED_NC
    if _CACHED_NC is None:
        _CACHED_NC = build_nc()
    return _CACHED_NC


def kernel(query, key, value, W_q, W_k, W_v, W_o, b_o):
    query = np.asarray(query, dtype=np.float32)
    key = np.asarray(key, dtype=np.float32)
    value = np.asarray(value, dtype=np.float32)
    W_q = np.asarray(W_q, dtype=np.float32)
    W_k = np.asarray(W_k, dtype=np.float32)
    W_v = np.asarray(W_v, dtype=np.float32)
    W_o = np.asarray(W_o, dtype=np.float32)
    b_o = np.asarray(b_o, dtype=np.float32)

    nc = _get_nc()
    in_maps = _make_in_maps(query, key, value, W_q, W_k, W_v, W_o, b_o)
    res = bass_utils.run_bass_kernel_spmd(nc, in_maps, core_ids=list(range(N_CORES)))

    attn = np.empty((B, H_TOT, S, S), dtype=np.float32)
    out = np.zeros((B, S, D), dtype=np.float32)
    for c in range(N_CORES):
        b = c // 4
        h0 = (c % 4) * H
        r = res.results[c]
        ptc = r["pt"]  # [H, NQH, S(k), QH(q)]
        for j in range(H):
            attn[b, h0 + j] = np.concatenate([ptc[j, q] for q in range(NQH)], axis=1).T
        out[b] += r["yt"].T  # [D, S] -> [S, D]
    return out, attn


# revision 35
# speedup vs baseline: 1.0978x; 1.0268x over previous
"""Multi-head attention Bass kernel for Trainium2, sharded over 8 NeuronCores.

Problem: B=2, S=2048, D=768, H=12 heads (d_k=64). Returns (output, attention_weights).

Sharding (data + head parallel): core c handles batch b = c//4 and heads
h0 = (c%4)*3 .. h0+3 (3 heads). W_q/W_k/W_v are split column-wise, W_o row-wise
over heads. Each core computes its 3 heads' [S,S] attention weights and a partial
output projection; the host sums the 4 partial outputs per batch and re-transposes
the attention weights.

Device-side layout: everything is computed in transposed orientation
(scores^T = [k, q]) so the second attention matmul (P @ V) needs no on-chip
transposes: lhsT = V_aug [k, d+1] (ones column appended -> sum(exp) for free),
rhs = E^T [k, q]. Attention weights are written to DRAM as P^T [h, k, q] and
un-transposed on the host during gather.

The scores matmuls contract over d_k=64 (half the PE array), so two heads'
scores run CONCURRENTLY via row-group packing (lhsT base partitions 0 / 64).
Heads are processed as unit0 = head0 (solo, minimizes time to first output)
then unit1 = head1+head2 packed.

Precision: matmul inputs are bf16 (PE full rate); accumulation is fp32 in PSUM;
softmax (exp, reciprocal, normalize) is fp32; outputs are fp32.
"""
import numpy as np
import ml_dtypes

import concourse.bass as bass
import concourse.tile as tile
from concourse import bacc, mybir, bass_utils

F32 = mybir.dt.float32
BF16 = mybir.dt.bfloat16
AF = mybir.ActivationFunctionType

B = 2
S = 2048
D = 768
H_TOT = 12
DK = 64
H = 3            # heads per core
N_CORES = 8
ST = S // 128    # 16 seq tiles
DT = D // 128    # 6 d-model tiles
QH = 1024        # q-half size
NQH = S // QH    # 2
SCALE = 1.0 / np.sqrt(DK)

_CACHED_NC = None


def build_nc():
    nc = bacc.Bacc("TRN2", target_bir_lowering=False, debug=False, num_devices=N_CORES)

    xq = nc.dram_tensor("xq", [D, S], BF16, kind="ExternalInput").ap()  # x^T, host-prep
    xk = nc.dram_tensor("xk", [D, S], BF16, kind="ExternalInput").ap()
    xv = nc.dram_tensor("xv", [D, S], BF16, kind="ExternalInput").ap()
    wq = nc.dram_tensor("wq", [D, H * DK], BF16, kind="ExternalInput").ap()
    wk = nc.dram_tensor("wk", [D, H * DK], BF16, kind="ExternalInput").ap()
    wv = nc.dram_tensor("wv", [D, H * DK], BF16, kind="ExternalInput").ap()
    wo = nc.dram_tensor("wo", [H * DK, D], BF16, kind="ExternalInput").ap()
    bo = nc.dram_tensor("bo", [D], F32, kind="ExternalInput").ap()

    pt = nc.dram_tensor("pt", [H, NQH, S, QH], F32, kind="ExternalOutput").ap()
    yt = nc.dram_tensor("yt", [D, S], F32, kind="ExternalOutput").ap()

    with tile.TileContext(nc) as tc:
        _emit(nc, tc, xq, xk, xv, wq, wk, wv, wo, bo, pt, yt)
    nc.compile()
    return nc


def _emit(nc, tc, xq, xk, xv, wq, wk, wv, wo, bo, pt, yt):
    from contextlib import ExitStack

    ctx = ExitStack()
    singles = ctx.enter_context(tc.tile_pool(name="singles", bufs=1))
    # Q^T / K^T head-packed tiles: tile 0 = heads 0,1 (partitions 0-63 / 64-127),
    # tile 1 = head 2 (partitions 0-63).
    qkt_pool = ctx.enter_context(tc.tile_pool(name="qkt", bufs=1))
    vaug_pool = ctx.enter_context(tc.tile_pool(name="vaug", bufs=1))
    an_pool = ctx.enter_context(tc.tile_pool(name="an", bufs=1))
    et_pool = ctx.enter_context(tc.tile_pool(name="et", bufs=1))
    small_pool = ctx.enter_context(tc.tile_pool(name="small", bufs=2))
    rbc_pool = ctx.enter_context(tc.tile_pool(name="rbc", bufs=3))
    rdram_pool = ctx.enter_context(tc.tile_pool(name="rdram", bufs=3, space="DRAM"))
    ptn_pool = ctx.enter_context(tc.tile_pool(name="ptn", bufs=1))
    eta_pool = ctx.enter_context(tc.tile_pool(name="eta", bufs=1))
    y_pool = ctx.enter_context(tc.tile_pool(name="ysb", bufs=2))
    ps_s = ctx.enter_context(tc.tile_pool(name="ps_s", bufs=2, space="PSUM"))
    ps_o = ctx.enter_context(tc.tile_pool(name="ps_o", bufs=2, space="PSUM"))

    ones_bf = singles.tile([128, 1], BF16)
    nc.vector.memset(ones_bf[:], 1.0)
    wo12_t = singles.tile([128, D], BF16)      # W_o rows 64..191 (heads 1,2)
    nc.sync.dma_start(wo12_t[:], wo[64 : H * DK, :])
    wo0_t = singles.tile([64, D], BF16)        # W_o rows 0..63 (head 0)
    nc.sync.dma_start(wo0_t[:], wo[0:64, :])
    bo_t = singles.tile([128, DT], F32)
    nc.sync.dma_start(bo_t[:], bo.rearrange("(t p) -> p t", p=128))

    qt_tiles = [qkt_pool.tile([128, S], BF16, tag=f"qt{m}", name=f"qt{m}") for m in range(2)]
    kt_tiles = [qkt_pool.tile([128, S], BF16, tag=f"kt{m}", name=f"kt{m}") for m in range(2)]
    vaug = [vaug_pool.tile([128, H, DK + 1], BF16, tag=f"va{m}", name=f"va{m}") for m in range(ST)]
    # an12: heads 1 (parts 0-63) and 2 (parts 64-127) packed; an0: head 0.
    an12 = [an_pool.tile([128, QH], BF16, tag=f"an12_{q}", name=f"an12_{q}") for q in range(NQH)]
    an0 = [an_pool.tile([DK, QH], BF16, tag=f"an0_{q}", name=f"an0_{q}") for q in range(NQH)]

    # head -> (tile, base partition) in the packed Q^T/K^T tiles
    HSLOT = {0: (0, 0), 1: (0, 64), 2: (1, 0)}

    # ---------------- Phase A: loads + projections + first unit ----------------
    def make_attn_unit(et_lookup):
        def attn_unit(qh, heads, lanes):
            o_ps = {}
            for h in heads:
                o_ps[h] = ps_o.tile([DK + 1, QH], F32, tag="ops", name=f"o_ps{h}")
            ets = {h: [] for h in heads}
            for kt in range(ST):
                s_tiles = {}
                for h in heads:
                    s_tiles[h] = ps_s.tile([128, QH], F32, tag="sps", name=f"s_ps{h}")
                for j in range(QH // 512):
                    for h in heads:
                        ti, p0 = HSLOT[h]
                        nc.tensor.matmul(
                            s_tiles[h][:, j * 512 : (j + 1) * 512],
                            kt_tiles[ti][p0 : p0 + DK, kt * 128 : (kt + 1) * 128],
                            qt_tiles[ti][p0 : p0 + DK,
                                         qh * QH + j * 512 : qh * QH + (j + 1) * 512],
                            start=True,
                            stop=True,
                            tile_position=(p0, 0),
                        )
                for i, h in enumerate(heads):
                    pool, lane = et_lookup(lanes[i])
                    et = pool.tile([128, QH], BF16, tag=f"et{lane}_{kt}",
                                   name=f"et{lane}_{kt}")
                    ets[h].append(et)
                    nc.scalar.activation(et[:], s_tiles[h][:], AF.Exp, scale=float(SCALE))
                for h in heads:
                    et = ets[h][kt]
                    for j in range(QH // 512):
                        nc.tensor.matmul(
                            o_ps[h][:, j * 512 : (j + 1) * 512],
                            vaug[kt][:, h, :],
                            et[:, j * 512 : (j + 1) * 512],
                            start=(kt == 0),
                            stop=(kt == ST - 1),
                        )

            rbcs = {}
            for h in heads:
                # free o_ps fast via ACT copy; recip spread over 128 partitions;
                # all small DMAs on gpsimd SWDGE (off the P^T sync FIFO)
                o_sb = small_pool.tile([DK + 1, QH], F32, tag="osb", name="o_sb")
                nc.scalar.copy(o_sb[:], o_ps[h][:])
                r_d = rdram_pool.tile([1, QH], F32, tag="rd", name="r_d")
                nc.sync.dma_start(r_d[:], o_sb[DK : DK + 1, :])
                rs8 = small_pool.tile([128, QH // 128], F32, tag="rs8", name="rs8")
                nc.sync.dma_start(rs8[:], r_d[:].rearrange("a (p e) -> (a p) e", p=128))
                nc.vector.reciprocal(rs8[:], rs8[:])
                r2_d = rdram_pool.tile([1, QH], F32, tag="rd2", name="r2_d")
                nc.sync.dma_start(r2_d[:].rearrange("a (p e) -> (a p) e", p=128), rs8[:])
                rbc = rbc_pool.tile([128, QH], F32, tag="rbc", name="rbc")
                nc.sync.dma_start(rbc[:], r2_d[:].to_broadcast((128, QH)))
                rbc_bf = rbc_pool.tile([128, QH], BF16, tag="rbcb", name="rbc_bf")
                nc.vector.tensor_copy(rbc_bf[:], rbc[:])
                rbcs[h] = rbc_bf

                if h == 0:
                    an_dst = an0[qh][:]
                else:
                    an_dst = an12[qh][(h - 1) * DK : h * DK, :]
                nc.vector.tensor_mul(an_dst, o_sb[0:DK, :], rbc[0:DK, :])

            for h in heads:
                rbc_bf = rbcs[h]
                for kt in range(ST):
                    ptt = ptn_pool.tile([128, QH], BF16, tag=f"ptn{kt % 4}",
                                        name=f"ptn_{kt % 4}")
                    nc.vector.tensor_mul(ptt[:], ets[h][kt][:], rbc_bf[:])
                    nc.gpsimd.dma_start(pt[h, qh, kt * 128 : (kt + 1) * 128, :], ptt[:])
        return attn_unit

    with tc.tile_pool(name="xt", bufs=2) as xt_pool, \
         tc.tile_pool(name="wqkv", bufs=1) as w_pool:
        wq_t = w_pool.tile([128, DT, H * DK], BF16)
        wk_t = w_pool.tile([128, DT, H * DK], BF16)
        wv_t = w_pool.tile([128, DT, H * DK], BF16)
        nc.sync.dma_start(wq_t[:], wq.rearrange("(t p) n -> p t n", p=128))
        nc.sync.dma_start(wk_t[:], wk.rearrange("(t p) n -> p t n", p=128))
        nc.sync.dma_start(wv_t[:], wv.rearrange("(t p) n -> p t n", p=128))

        xts = {}
        for which, xin in ((2, xv), (0, xq), (1, xk)):
            xin_r = xin.rearrange("(t p) s -> p t s", p=128)
            xt = xt_pool.tile([128, DT, S], BF16, tag="xt", name=f"xt{which}")
            xts[which] = xt
            for dt in range(DT):
                nc.sync.dma_start(xt[:, dt, :], xin_r[:, dt, :])

        # ---- V projection -> V_aug [k, (h, d+1)] with ones column ----
        for m in range(ST):
            pv = ps_s.tile([128, QH], F32, tag="sps", name="pv")
            for dt in range(DT):
                nc.tensor.matmul(
                    pv[:, 0 : H * DK],
                    xts[2][:, dt, m * 128 : (m + 1) * 128],
                    wv_t[:, dt, :],
                    start=(dt == 0),
                    stop=(dt == DT - 1),
                )
            va = vaug[m]
            nc.vector.tensor_copy(
                va[:, :, 0:DK],
                pv[:, 0 : H * DK].rearrange("p (h d) -> p h d", h=H),
            )
            nc.vector.tensor_copy(va[:, :, DK : DK + 1], ones_bf[:].to_broadcast((128, H, 1)))

        def project(which, m):
            xt = xts[which]
            w_t = wq_t if which == 0 else wk_t
            dst = (qt_tiles if which == 0 else kt_tiles)[m]
            cols = slice(m * 128, m * 128 + (128 if m == 0 else 64))
            npart = 128 if m == 0 else 64
            for jh in range(NQH):
                pp = ps_s.tile([128, QH], F32, tag="sps", name="pp")
                for dt in range(DT):
                    for j in range(QH // 512):
                        nc.tensor.matmul(
                            pp[:npart, j * 512 : (j + 1) * 512],
                            w_t[:, dt, cols],
                            xt[:, dt, jh * QH + j * 512 : jh * QH + (j + 1) * 512],
                            start=(dt == 0),
                            stop=(dt == DT - 1),
                        )
                nc.vector.tensor_copy(dst[:npart, jh * QH : (jh + 1) * QH], pp[:npart, :])

        project(0, 0)
        project(1, 0)

        attn_unit_a = make_attn_unit(lambda lane: (eta_pool, lane))
        attn_unit_a(0, [0], ["a"])

        project(0, 1)
        project(1, 1)

    # ---------------- Phase B: remaining units + output projection -------------
    with tc.tile_pool(name="et", bufs=1) as et_pool:
        attn_unit = make_attn_unit(
            lambda lane: (eta_pool, lane) if lane == "a" else (et_pool, lane)
        )

        def emit_y(qh):
            for dt in range(DT):
                y_ps = ps_o.tile([128, QH], F32, tag="ops", name="y_ps")
                for j in range(QH // 512):
                    js = slice(j * 512, (j + 1) * 512)
                    nc.tensor.matmul(
                        y_ps[:, js],
                        wo12_t[:, dt * 128 : (dt + 1) * 128],
                        an12[qh][:, js],
                        start=True,
                        stop=False,
                    )
                    nc.tensor.matmul(
                        y_ps[:, js],
                        wo0_t[:, dt * 128 : (dt + 1) * 128],
                        an0[qh][:, js],
                        start=False,
                        stop=True,
                    )
                y_sb = y_pool.tile([128, QH], F32, tag="y", name="y_sb")
                nc.scalar.activation(
                    y_sb[:], y_ps[:], AF.Identity, bias=bo_t[:, dt : dt + 1], scale=1.0
                )
                nc.scalar.dma_start(
                    yt[dt * 128 : (dt + 1) * 128, qh * QH : (qh + 1) * QH], y_sb[:]
                )

        attn_unit(0, [1, 2], [0, 1])
        attn_unit(1, [1, 2], ["a", 2])
        emit_y(0)
        attn_unit(1, [0], [1])
        emit_y(1)

    ctx.close()


def _make_in_maps(query, key, value, W_q, W_k, W_v, W_o, b_o):
    bf = ml_dtypes.bfloat16
    in_maps = []
    for c in range(N_CORES):
        b = c // 4
        h0 = (c % 4) * H
        cols = slice(h0 * DK, (h0 + H) * DK)
        in_maps.append(
            {
                "xq": np.ascontiguousarray(query[b].T).astype(bf),
                "xk": np.ascontiguousarray(key[b].T).astype(bf),
                "xv": np.ascontiguousarray(value[b].T).astype(bf),
                "wq": np.ascontiguousarray(W_q[:, cols]).astype(bf),
                "wk": np.ascontiguousarray(W_k[:, cols]).astype(bf),
                "wv": np.ascontiguousarray(W_v[:, cols]).astype(bf),
                "wo": np.ascontiguousarray(W_o[cols, :]).astype(bf),
                "bo": np.ascontiguousarray(b_o) / 4.0,
            }
        )
    return in_maps


def run_traced(inputs):
    """Run with NTFF tracing to get HW exec time (test-only helper)."""
    nc = _get_nc()
    in_maps = _make_in_maps(
        inputs["query"], inputs["key"], inputs["value"],
        inputs["W_q"], inputs["W_k"], inputs["W_v"], inputs["W_o"], inputs["b_o"],
    )
    return bass_utils.run_bass_kernel_spmd(
        nc, in_maps, core_ids=list(range(N_CORES)), trace=True
    )


def _get_nc():
    global _CACHED_NC
    if _CACHED_NC is None:
        _CACHED_NC = build_nc()
    return _CACHED_NC


def kernel(query, key, value, W_q, W_k, W_v, W_o, b_o):
    query = np.asarray(query, dtype=np.float32)
    key = np.asarray(key, dtype=np.float32)
    value = np.asarray(value, dtype=np.float32)
    W_q = np.asarray(W_q, dtype=np.float32)
    W_k = np.asarray(W_k, dtype=np.float32)
    W_v = np.asarray(W_v, dtype=np.float32)
    W_o = np.asarray(W_o, dtype=np.float32)
    b_o = np.asarray(b_o, dtype=np.float32)

    nc = _get_nc()
    in_maps = _make_in_maps(query, key, value, W_q, W_k, W_v, W_o, b_o)
    res = bass_utils.run_bass_kernel_spmd(nc, in_maps, core_ids=list(range(N_CORES)))

    attn = np.empty((B, H_TOT, S, S), dtype=np.float32)
    out = np.zeros((B, S, D), dtype=np.float32)
    for c in range(N_CORES):
        b = c // 4
        h0 = (c % 4) * H
        r = res.results[c]
        ptc = r["pt"]  # [H, NQH, S(k), QH(q)]
        for j in range(H):
            attn[b, h0 + j] = np.concatenate([ptc[j, q] for q in range(NQH)], axis=1).T
        out[b] += r["yt"].T  # [D, S] -> [S, D]
    return out, attn


# revision 36
# speedup vs baseline: 1.1286x; 1.0281x over previous
"""Multi-head attention Bass kernel for Trainium2, sharded over 8 NeuronCores.

Problem: B=2, S=2048, D=768, H=12 heads (d_k=64). Returns (output, attention_weights).

Sharding (data + head parallel): core c handles batch b = c//4 and heads
h0 = (c%4)*3 .. h0+3 (3 heads). W_q/W_k/W_v are split column-wise, W_o row-wise
over heads. Each core computes its 3 heads' [S,S] attention weights and a partial
output projection; the host sums the 4 partial outputs per batch and re-transposes
the attention weights.

Device-side layout: everything is computed in transposed orientation
(scores^T = [k, q]) so the second attention matmul (P @ V) needs no on-chip
transposes: lhsT = V_aug [k, d+1] (ones column appended -> sum(exp) for free),
rhs = E^T [k, q]. Attention weights are written to DRAM as P^T [h, k, q] and
un-transposed on the host during gather.

The scores matmuls contract over d_k=64 (half the PE array), so two heads'
scores run CONCURRENTLY via row-group packing (lhsT base partitions 0 / 64).
Heads are processed as unit0 = head0 (solo, minimizes time to first output)
then unit1 = head1+head2 packed.

Precision: matmul inputs are bf16 (PE full rate); accumulation is fp32 in PSUM;
softmax (exp, reciprocal, normalize) is fp32; outputs are fp32.
"""
import numpy as np
import ml_dtypes

import concourse.bass as bass
import concourse.tile as tile
from concourse import bacc, mybir, bass_utils

F32 = mybir.dt.float32
BF16 = mybir.dt.bfloat16
AF = mybir.ActivationFunctionType

B = 2
S = 2048
D = 768
H_TOT = 12
DK = 64
H = 3            # heads per core
N_CORES = 8
ST = S // 128    # 16 seq tiles
DT = D // 128    # 6 d-model tiles
QH = 1024        # q-half size
NQH = S // QH    # 2
SCALE = 1.0 / np.sqrt(DK)

_CACHED_NC = None


def build_nc():
    nc = bacc.Bacc("TRN2", target_bir_lowering=False, debug=False, num_devices=N_CORES)

    xq = nc.dram_tensor("xq", [D, S], BF16, kind="ExternalInput").ap()  # x^T, host-prep
    xk = nc.dram_tensor("xk", [D, S], BF16, kind="ExternalInput").ap()
    xv = nc.dram_tensor("xv", [D, S], BF16, kind="ExternalInput").ap()
    wq = nc.dram_tensor("wq", [D, H * DK], BF16, kind="ExternalInput").ap()
    wk = nc.dram_tensor("wk", [D, H * DK], BF16, kind="ExternalInput").ap()
    wv = nc.dram_tensor("wv", [D, H * DK], BF16, kind="ExternalInput").ap()
    wo = nc.dram_tensor("wo", [H * DK, D], BF16, kind="ExternalInput").ap()
    bo = nc.dram_tensor("bo", [D], F32, kind="ExternalInput").ap()

    pt = nc.dram_tensor("pt", [H, NQH, S, QH], F32, kind="ExternalOutput").ap()
    yt = nc.dram_tensor("yt", [D, S], F32, kind="ExternalOutput").ap()

    with tile.TileContext(nc) as tc:
        _emit(nc, tc, xq, xk, xv, wq, wk, wv, wo, bo, pt, yt)
    nc.compile()
    return nc


def _emit(nc, tc, xq, xk, xv, wq, wk, wv, wo, bo, pt, yt):
    from contextlib import ExitStack

    ctx = ExitStack()
    singles = ctx.enter_context(tc.tile_pool(name="singles", bufs=1))
    # Q^T / K^T head-packed tiles: tile 0 = heads 0,1 (partitions 0-63 / 64-127),
    # tile 1 = head 2 (partitions 0-63).
    qkt_pool = ctx.enter_context(tc.tile_pool(name="qkt", bufs=1))
    vaug_pool = ctx.enter_context(tc.tile_pool(name="vaug", bufs=1))
    an_pool = ctx.enter_context(tc.tile_pool(name="an", bufs=1))
    et_pool = ctx.enter_context(tc.tile_pool(name="et", bufs=1))
    small_pool = ctx.enter_context(tc.tile_pool(name="small", bufs=2))
    rbc_pool = ctx.enter_context(tc.tile_pool(name="rbc", bufs=3))
    rdram_pool = ctx.enter_context(tc.tile_pool(name="rdram", bufs=3, space="DRAM"))
    ptn_pool = ctx.enter_context(tc.tile_pool(name="ptn", bufs=1))
    eta_pool = ctx.enter_context(tc.tile_pool(name="eta", bufs=1))
    y_pool = ctx.enter_context(tc.tile_pool(name="ysb", bufs=2))
    ps_s = ctx.enter_context(tc.tile_pool(name="ps_s", bufs=2, space="PSUM"))
    ps_o = ctx.enter_context(tc.tile_pool(name="ps_o", bufs=2, space="PSUM"))

    ones_bf = singles.tile([128, 1], BF16)
    nc.vector.memset(ones_bf[:], 1.0)
    wo12_t = singles.tile([128, D], BF16)      # W_o rows 64..191 (heads 1,2)
    nc.sync.dma_start(wo12_t[:], wo[64 : H * DK, :])
    wo0_t = singles.tile([64, D], BF16)        # W_o rows 0..63 (head 0)
    nc.sync.dma_start(wo0_t[:], wo[0:64, :])
    bo_t = singles.tile([128, DT], F32)
    nc.sync.dma_start(bo_t[:], bo.rearrange("(t p) -> p t", p=128))

    qt_tiles = [qkt_pool.tile([128, S], BF16, tag=f"qt{m}", name=f"qt{m}") for m in range(2)]
    kt_tiles = [qkt_pool.tile([128, S], BF16, tag=f"kt{m}", name=f"kt{m}") for m in range(2)]
    vaug = [vaug_pool.tile([128, H, DK + 1], BF16, tag=f"va{m}", name=f"va{m}") for m in range(ST)]
    # an12: heads 1 (parts 0-63) and 2 (parts 64-127) packed; an0: head 0.
    an12 = [an_pool.tile([128, QH], BF16, tag=f"an12_{q}", name=f"an12_{q}") for q in range(NQH)]
    an0 = [an_pool.tile([DK, QH], BF16, tag=f"an0_{q}", name=f"an0_{q}") for q in range(NQH)]

    # head -> (tile, base partition) in the packed Q^T/K^T tiles
    HSLOT = {0: (0, 0), 1: (0, 64), 2: (1, 0)}

    # ---------------- Phase A: loads + projections + first unit ----------------
    def make_attn_unit(et_lookup):
        def attn_unit(qh, heads, lanes):
            o_ps = {}
            for h in heads:
                o_ps[h] = ps_o.tile([DK + 1, QH], F32, tag="ops", name=f"o_ps{h}")
            ets = {h: [] for h in heads}
            for kt in range(ST):
                s_tiles = {}
                for h in heads:
                    s_tiles[h] = ps_s.tile([128, QH], F32, tag="sps", name=f"s_ps{h}")
                for j in range(QH // 512):
                    for h in heads:
                        ti, p0 = HSLOT[h]
                        nc.tensor.matmul(
                            s_tiles[h][:, j * 512 : (j + 1) * 512],
                            kt_tiles[ti][p0 : p0 + DK, kt * 128 : (kt + 1) * 128],
                            qt_tiles[ti][p0 : p0 + DK,
                                         qh * QH + j * 512 : qh * QH + (j + 1) * 512],
                            start=True,
                            stop=True,
                            tile_position=(p0, 0),
                        )
                for i, h in enumerate(heads):
                    pool, lane = et_lookup(lanes[i])
                    et = pool.tile([128, QH], BF16, tag=f"et{lane}_{kt}",
                                   name=f"et{lane}_{kt}")
                    ets[h].append(et)
                    nc.scalar.activation(et[:], s_tiles[h][:], AF.Exp, scale=float(SCALE))
                for h in heads:
                    et = ets[h][kt]
                    for j in range(QH // 512):
                        nc.tensor.matmul(
                            o_ps[h][:, j * 512 : (j + 1) * 512],
                            vaug[kt][:, h, :],
                            et[:, j * 512 : (j + 1) * 512],
                            start=(kt == 0),
                            stop=(kt == ST - 1),
                        )

            rbcs = {}
            for h in heads:
                # free o_ps fast via ACT copy; recip spread over 128 partitions;
                # all small DMAs on gpsimd SWDGE (off the P^T sync FIFO)
                o_sb = small_pool.tile([DK + 1, QH], F32, tag="osb", name="o_sb")
                nc.scalar.copy(o_sb[:], o_ps[h][:])
                r_d = rdram_pool.tile([1, QH], F32, tag="rd", name="r_d")
                nc.sync.dma_start(r_d[:], o_sb[DK : DK + 1, :])
                rs8 = small_pool.tile([128, QH // 128], F32, tag="rs8", name="rs8")
                nc.sync.dma_start(rs8[:], r_d[:].rearrange("a (p e) -> (a p) e", p=128))
                nc.vector.reciprocal(rs8[:], rs8[:])
                r2_d = rdram_pool.tile([1, QH], F32, tag="rd2", name="r2_d")
                nc.sync.dma_start(r2_d[:].rearrange("a (p e) -> (a p) e", p=128), rs8[:])
                rbc = rbc_pool.tile([128, QH], F32, tag="rbc", name="rbc")
                nc.sync.dma_start(rbc[:], r2_d[:].to_broadcast((128, QH)))
                rbc_bf = rbc_pool.tile([128, QH], BF16, tag="rbcb", name="rbc_bf")
                nc.vector.tensor_copy(rbc_bf[:], rbc[:])
                rbcs[h] = rbc_bf

                if h == 0:
                    an_dst = an0[qh][:]
                else:
                    an_dst = an12[qh][(h - 1) * DK : h * DK, :]
                nc.vector.tensor_mul(an_dst, o_sb[0:DK, :], rbc[0:DK, :])

            for h in heads:
                rbc_bf = rbcs[h]
                for kt in range(ST):
                    ptt = ptn_pool.tile([128, QH], BF16, tag=f"ptn{kt % 6}",
                                        name=f"ptn_{kt % 6}")
                    nc.vector.tensor_mul(ptt[:], ets[h][kt][:], rbc_bf[:])
                    nc.gpsimd.dma_start(pt[h, qh, kt * 128 : (kt + 1) * 128, :], ptt[:])
        return attn_unit

    with tc.tile_pool(name="xt", bufs=2) as xt_pool, \
         tc.tile_pool(name="wqkv", bufs=1) as w_pool:
        wq_t = w_pool.tile([128, DT, H * DK], BF16)
        wk_t = w_pool.tile([128, DT, H * DK], BF16)
        wv_t = w_pool.tile([128, DT, H * DK], BF16)
        nc.sync.dma_start(wq_t[:], wq.rearrange("(t p) n -> p t n", p=128))
        nc.sync.dma_start(wk_t[:], wk.rearrange("(t p) n -> p t n", p=128))
        nc.sync.dma_start(wv_t[:], wv.rearrange("(t p) n -> p t n", p=128))

        xts = {}
        for which, xin in ((2, xv), (0, xq), (1, xk)):
            xin_r = xin.rearrange("(t p) s -> p t s", p=128)
            xt = xt_pool.tile([128, DT, S], BF16, tag="xt", name=f"xt{which}")
            xts[which] = xt
            for dt in range(DT):
                nc.sync.dma_start(xt[:, dt, :], xin_r[:, dt, :])

        # ---- V projection -> V_aug [k, (h, d+1)] with ones column ----
        for m in range(ST):
            pv = ps_s.tile([128, QH], F32, tag="sps", name="pv")
            for dt in range(DT):
                nc.tensor.matmul(
                    pv[:, 0 : H * DK],
                    xts[2][:, dt, m * 128 : (m + 1) * 128],
                    wv_t[:, dt, :],
                    start=(dt == 0),
                    stop=(dt == DT - 1),
                )
            va = vaug[m]
            nc.vector.tensor_copy(
                va[:, :, 0:DK],
                pv[:, 0 : H * DK].rearrange("p (h d) -> p h d", h=H),
            )
            nc.vector.tensor_copy(va[:, :, DK : DK + 1], ones_bf[:].to_broadcast((128, H, 1)))

        def project(which, m):
            xt = xts[which]
            w_t = wq_t if which == 0 else wk_t
            dst = (qt_tiles if which == 0 else kt_tiles)[m]
            cols = slice(m * 128, m * 128 + (128 if m == 0 else 64))
            npart = 128 if m == 0 else 64
            for jh in range(NQH):
                pp = ps_s.tile([128, QH], F32, tag="sps", name="pp")
                for dt in range(DT):
                    for j in range(QH // 512):
                        nc.tensor.matmul(
                            pp[:npart, j * 512 : (j + 1) * 512],
                            w_t[:, dt, cols],
                            xt[:, dt, jh * QH + j * 512 : jh * QH + (j + 1) * 512],
                            start=(dt == 0),
                            stop=(dt == DT - 1),
                        )
                nc.vector.tensor_copy(dst[:npart, jh * QH : (jh + 1) * QH], pp[:npart, :])

        project(0, 0)
        project(1, 0)

        attn_unit_a = make_attn_unit(lambda lane: (eta_pool, lane))
        attn_unit_a(0, [0], ["a"])

        project(0, 1)
        project(1, 1)

    # ---------------- Phase B: remaining units + output projection -------------
    with tc.tile_pool(name="et", bufs=1) as et_pool:
        attn_unit = make_attn_unit(
            lambda lane: (eta_pool, lane) if lane == "a" else (et_pool, lane)
        )

        def emit_y(qh):
            for dt in range(DT):
                y_ps = ps_o.tile([128, QH], F32, tag="ops", name="y_ps")
                for j in range(QH // 512):
                    js = slice(j * 512, (j + 1) * 512)
                    nc.tensor.matmul(
                        y_ps[:, js],
                        wo12_t[:, dt * 128 : (dt + 1) * 128],
                        an12[qh][:, js],
                        start=True,
                        stop=False,
                    )
                    nc.tensor.matmul(
                        y_ps[:, js],
                        wo0_t[:, dt * 128 : (dt + 1) * 128],
                        an0[qh][:, js],
                        start=False,
                        stop=True,
                    )
                y_sb = y_pool.tile([128, QH], F32, tag="y", name="y_sb")
                nc.scalar.activation(
                    y_sb[:], y_ps[:], AF.Identity, bias=bo_t[:, dt : dt + 1], scale=1.0
                )
                nc.scalar.dma_start(
                    yt[dt * 128 : (dt + 1) * 128, qh * QH : (qh + 1) * QH], y_sb[:]
                )

        attn_unit(0, [1, 2], [0, 1])
        attn_unit(1, [1, 2], ["a", 2])
        emit_y(0)
        attn_unit(1, [0], [1])
        emit_y(1)

    ctx.close()


def _make_in_maps(query, key, value, W_q, W_k, W_v, W_o, b_o):
    bf = ml_dtypes.bfloat16
    in_maps = []
    for c in range(N_CORES):
        b = c // 4
        h0 = (c % 4) * H
        cols = slice(h0 * DK, (h0 + H) * DK)
        in_maps.append(
            {
                "xq": np.ascontiguousarray(query[b].T).astype(bf),
                "xk": np.ascontiguousarray(key[b].T).astype(bf),
                "xv": np.ascontiguousarray(value[b].T).astype(bf),
                "wq": np.ascontiguousarray(W_q[:, cols]).astype(bf),
                "wk": np.ascontiguousarray(W_k[:, cols]).astype(bf),
                "wv": np.ascontiguousarray(W_v[:, cols]).astype(bf),
                "wo": np.ascontiguousarray(W_o[cols, :]).astype(bf),
                "bo": np.ascontiguousarray(b_o) / 4.0,
            }
        )
    return in_maps


def run_traced(inputs):
    """Run with NTFF tracing to get HW exec time (test-only helper)."""
    nc = _get_nc()
    in_maps = _make_in_maps(
        inputs["query"], inputs["key"], inputs["value"],
        inputs["W_q"], inputs["W_k"], inputs["W_v"], inputs["W_o"], inputs["b_o"],
    )
    return bass_utils.run_bass_kernel_spmd(
        nc, in_maps, core_ids=list(range(N_CORES)), trace=True
    )


def _get_nc():
    global _CACHED_NC
    if _CACHED_NC is None:
        _CACHED_NC = build_nc()
    return _CACHED_NC


def kernel(query, key, value, W_q, W_k, W_v, W_o, b_o):
    query = np.asarray(query, dtype=np.float32)
    key = np.asarray(key, dtype=np.float32)
    value = np.asarray(value, dtype=np.float32)
    W_q = np.asarray(W_q, dtype=np.float32)
    W_k = np.asarray(W_k, dtype=np.float32)
    W_v = np.asarray(W_v, dtype=np.float32)
    W_o = np.asarray(W_o, dtype=np.float32)
    b_o = np.asarray(b_o, dtype=np.float32)

    nc = _get_nc()
    in_maps = _make_in_maps(query, key, value, W_q, W_k, W_v, W_o, b_o)
    res = bass_utils.run_bass_kernel_spmd(nc, in_maps, core_ids=list(range(N_CORES)))

    attn = np.empty((B, H_TOT, S, S), dtype=np.float32)
    out = np.zeros((B, S, D), dtype=np.float32)
    for c in range(N_CORES):
        b = c // 4
        h0 = (c % 4) * H
        r = res.results[c]
        ptc = r["pt"]  # [H, NQH, S(k), QH(q)]
        for j in range(H):
            attn[b, h0 + j] = np.concatenate([ptc[j, q] for q in range(NQH)], axis=1).T
        out[b] += r["yt"].T  # [D, S] -> [S, D]
    return out, attn
